# revision 1
# baseline (speedup 1.0000x reference)
"""Complex-valued causal attention on 8 trn2 NeuronCores.

nn_ComplexAttention: B=2, L=2048, D=1024, H=16 heads (hd=64), fp32 I/O.

Sharding (per the batch+head hint): core c owns batch b = c//4 and heads
4*(c%4) .. 4*(c%4)+3.  Data parallel over B (2 groups of 4 cores), tensor
parallel over heads within a group.  After per-head attention the 4 cores of
a group AllGather the (d-major, fp16) attention outputs and each computes a
256-column slice of the output projection, so the only collective is a 2 MB
AllGather per core.

All on-chip math uses fp16 operands with fp32 PSUM accumulation (fp16 keeps
8 more mantissa bits than bf16 at the same PE/DVE/DMA cost).  Everything is
formulated transposed (d-major) so no activation transposes are needed:

  Qc^T[h] = Wq_eff^T @ xc^T        (xc^T = [x_real^T ; x_imag^T], host-prepped)
  S^T     = Kc^T-block^T @ Qc^T    (real part of complex dot product, both
                                    r/i folded into the 128-deep contraction)
  w^T     = exp(SCALE * S^T)       (no max-subtraction needed: |scores| <~ 8)
  O^T     = V-block^T @ w^T        (V seq-major via 128x128 PE transposes)
  sums    = ones^T @ w^T           (softmax denominators via matmul)
  y^T     = Wo_eff^T @ oc^T        (oc^T = AllGather of all heads' O^T)

The complex arithmetic (4 real matmuls per complex one) is folded into the
host-assembled W_eff matrices with +-W_r/W_i blocks.
"""

import sys

if "/opt/trn_rl_repo" not in sys.path:
    sys.path.insert(0, "/opt/trn_rl_repo")

import numpy as np
import ml_dtypes

import concourse.mybir as mybir
import concourse.tile as tile
from concourse import bacc
from concourse.bass_utils import run_bass_kernel_spmd

B, L, D, H = 2, 2048, 1024, 16
HD = D // H            # 64
SCALE = HD ** (-0.5)
NCORES = 8
GROUP = 4              # cores per batch group
NH = H // GROUP        # 4 local heads per core
JC = NH * 2 * HD       # 512 local projection cols (r+i interleaved by head)
DD = 2 * D             # 2048 stacked (real; imag) contraction dim
F16 = mybir.dt.float16
F32 = mybir.dt.float32

_CACHE = {}


def _build(seq_len=L, repeat=1, with_cc=True, phases="ABC", compile=True):
    """Build + compile the SPMD kernel (identical program on all 8 cores).

    repeat>1 wraps the whole body in a hardware For_i loop (timing variant,
    collective skipped since collectives cannot sit inside control flow).
    """
    from contextlib import nullcontext
    LL = seq_len
    NLC = LL // 512        # l-chunks of 512
    NKB = LL // 128        # k-blocks of 128
    NDD = DD // 128        # contraction chunks (16)

    nc = bacc.Bacc("TRN2", target_bir_lowering=False, debug=False,
                   num_devices=NCORES)

    xcT = nc.dram_tensor("xcT", [DD, LL], F16, kind="ExternalInput")
    wq = nc.dram_tensor("wq", [DD, JC], F16, kind="ExternalInput")
    wk = nc.dram_tensor("wk", [DD, JC], F16, kind="ExternalInput")
    wv = nc.dram_tensor("wv", [DD, JC], F16, kind="ExternalInput")
    wo = nc.dram_tensor("wo", [DD, JC], F16, kind="ExternalInput")
    bo = nc.dram_tensor("bo", [JC, 1], F32, kind="ExternalInput")
    mask = nc.dram_tensor("mask", [128, 128], F16, kind="ExternalInput")
    ident = nc.dram_tensor("ident", [128, 128], F16, kind="ExternalInput")
    ones = nc.dram_tensor("ones", [128, 1], F16, kind="ExternalInput")
    yT = nc.dram_tensor("yT", [JC, LL], F32, kind="ExternalOutput")

    with tile.TileContext(nc) as tc:
        with (
            tc.tile_pool(name="const", bufs=1) as const,
            tc.tile_pool(name="dram", bufs=1, space="DRAM") as dram,
        ):
            mask_t = const.tile([128, 128], F16, tag="mask", name="mask")
            nc.sync.dma_start(mask_t[:], mask[:])
            ident_t = const.tile([128, 128], F16, tag="ident", name="ident")
            nc.sync.dma_start(ident_t[:], ident[:])
            ones_t = const.tile([128, 1], F16, tag="ones", name="ones")
            nc.sync.dma_start(ones_t[:], ones[:])
            bo_t = const.tile([128, NH], F32, tag="bo", name="bo")
            for m in range(NH):
                nc.sync.dma_start(bo_t[:, m:m + 1], bo[m * 128:(m + 1) * 128, :])

            ag_in = dram.tile([JC, LL], F16, tag="ag_in", name="ag_in")
            ag_out = dram.tile([DD, LL], F16, tag="ag_out", name="ag_out")

            _hint = (mybir.EngineType.PE, mybir.EngineType.Activation,
                     mybir.EngineType.DVE, mybir.EngineType.SP,
                     mybir.EngineType.Pool)
            loop_cm = (tc.For_i(0, repeat, 1, hint_engines=_hint)
                       if repeat > 1 else nullcontext())
            with loop_cm:
                with (
                    tc.tile_pool(name="qkvout", bufs=1) as qkvout,
                    tc.tile_pool(name="vpool", bufs=1) as vpool,
                ):
                    qT = [qkvout.tile([128, LL], F16, tag=f"qT{h}", name=f"qT{h}") for h in range(NH)]
                    kT = [qkvout.tile([128, LL], F16, tag=f"kT{h}", name=f"kT{h}") for h in range(NH)]
                    vv = [vpool.tile([128, LL], F16, tag=f"v{h}", name=f"v{h}") for h in range(NH)]

                    # ---------------- Phase A: projections + V transpose ----------
                    with (
                        tc.tile_pool(name="xw", bufs=1) as xw,
                        tc.tile_pool(name="vt", bufs=1) as vt,
                        tc.tile_pool(name="pps", bufs=3, space="PSUM") as pps,
                        tc.tile_pool(name="vtps", bufs=2, space="PSUM") as vtps,
                    ):
                        xcT_t = []
                        for k in range(NDD):
                            t = xw.tile([128, LL], F16, tag=f"xcT{k}", name=f"xcT{k}")
                            nc.sync.dma_start(t[:], xcT[k * 128:(k + 1) * 128, :])
                            xcT_t.append(t)
                        w_tiles = {}
                        for wname, wext in (("wq", wq), ("wk", wk), ("wv", wv)):
                            tl = []
                            for k in range(NDD):
                                t = xw.tile([128, JC], F16, tag=f"{wname}{k}", name=f"{wname}{k}")
                                nc.sync.dma_start(t[:], wext[k * 128:(k + 1) * 128, :])
                                tl.append(t)
                            w_tiles[wname] = tl

                        vT = [vt.tile([128, LL], F16, tag=f"vT{h}", name=f"vT{h}") for h in range(NH)]

                        for h in range(NH):
                            for wname, outs in (("wq", qT), ("wk", kT), ("wv", vT)):
                                wt = w_tiles[wname]
                                for n in range(NLC):
                                    ps = pps.tile([128, 512], F32, tag="projps", name="projps")
                                    for k in range(NDD):
                                        nc.tensor.matmul(
                                            ps[:],
                                            wt[k][:, h * 128:(h + 1) * 128],
                                            xcT_t[k][:, n * 512:(n + 1) * 512],
                                            start=(k == 0), stop=(k == NDD - 1),
                                        )
                                    nc.vector.tensor_copy(
                                        outs[h][:, n * 512:(n + 1) * 512], ps[:])
                            # V^T[h] -> V[h] (seq-major) via PE 128x128
                            # transposes, 8 per PSUM bank + one batched copy
                            for g in range(NKB // 8):
                                tp = vtps.tile([128, 1024], F16, tag="vtp", name="vtp")
                                for j in range(8):
                                    kb = g * 8 + j
                                    nc.tensor.transpose(
                                        tp[:, j * 128:(j + 1) * 128],
                                        vT[h][:, kb * 128:(kb + 1) * 128], ident_t[:])
                                nc.vector.tensor_copy(
                                    vv[h][:, g * 1024:(g + 1) * 1024], tp[:])

                    if "B" in phases:
                        # ---------------- Phase B: flash attention per (head, q-chunk)
                        with (
                            tc.tile_pool(name="sps", bufs=4, space="PSUM") as sps,
                            tc.tile_pool(name="ops", bufs=2, space="PSUM") as ops,
                            tc.tile_pool(name="sums", bufs=2, space="PSUM") as sums,
                            tc.tile_pool(name="wpool", bufs=8) as wpool,
                            tc.tile_pool(name="bpool", bufs=2) as bpool,
                            tc.tile_pool(name="opool", bufs=2) as opool,
                        ):
                            for h in range(NH):
                                for qc in range(NLC):
                                    o_ps = ops.tile([128, 512], F32, tag="o", name="o")
                                    s_sum = sums.tile([1, 512], F32, tag="s", name="s")
                                    nkb = 4 * (qc + 1)
                                    # 1-stage software pipeline: issue the
                                    # next k-block's scores matmul before the
                                    # current block's ones/O matmuls so the PE
                                    # overlaps the exp on ScalarE.
                                    def consume(kb, f0, w, wx):
                                        nc.tensor.matmul(
                                            s_sum[0:1, f0:512], ones_t[:],
                                            wx[:, :w],
                                            start=(kb == 0), stop=(kb == nkb - 1))
                                        nc.tensor.matmul(
                                            o_ps[:, f0:512],
                                            vv[h][:, kb * 128:(kb + 1) * 128],
                                            wx[:, :w],
                                            start=(kb == 0), stop=(kb == nkb - 1))
                                    pend = None
                                    for kb in range(nkb):
                                        r = kb - 4 * qc
                                        f0 = 128 * r if r >= 0 else 0
                                        w = 512 - f0
                                        s_ps = sps.tile([128, 512], F32, tag="sc", name="sc")
                                        nc.tensor.matmul(
                                            s_ps[:, :w],
                                            kT[h][:, kb * 128:(kb + 1) * 128],
                                            qT[h][:, qc * 512 + f0: (qc + 1) * 512],
                                            start=True, stop=True)
                                        wx = wpool.tile([128, 512], F16, tag="wx", name="wx")
                                        nc.scalar.activation(
                                            wx[:, :w], s_ps[:, :w],
                                            mybir.ActivationFunctionType.Exp, scale=SCALE)
                                        if r >= 0:
                                            nc.vector.tensor_mul(
                                                wx[:, :128], wx[:, :128], mask_t[:])
                                        if pend is not None:
                                            consume(*pend)
                                        pend = (kb, f0, w, wx)
                                    consume(*pend)
                                    rec = bpool.tile([1, 512], F32, tag="rec", name="rec")
                                    nc.vector.reciprocal(rec[:], s_sum[:])
                                    bca = bpool.tile([128, 512], F32, tag="bca", name="bca")
                                    nc.gpsimd.partition_broadcast(bca[:], rec[:])
                                    osc = opool.tile([128, 512], F16, tag="osc", name="osc")
                                    nc.vector.tensor_mul(osc[:], o_ps[:], bca[:])
                                    nc.sync.dma_start(
                                        ag_in[h * 128:(h + 1) * 128,
                                              qc * 512:(qc + 1) * 512], osc[:])

                # ---------------- AllGather within each batch group --------------
                if with_cc and "C" in phases:
                    nc.gpsimd.collective_compute(
                        "AllGather",
                        mybir.AluOpType.bypass,
                        replica_groups=[[0, 1, 2, 3], [4, 5, 6, 7]],
                        ins=[ag_in.opt()],
                        outs=[ag_out.opt()],
                    )

                if "C" in phases:
                    # ---------------- Phase C: output projection ---------------------
                    with (
                        tc.tile_pool(name="ocp", bufs=1) as ocp,
                        tc.tile_pool(name="wop", bufs=1) as wop,
                        tc.tile_pool(name="yps", bufs=2, space="PSUM") as yps,
                        tc.tile_pool(name="ysb", bufs=3) as ysbp,
                    ):
                        oc_t = []
                        for k in range(NDD):
                            t = ocp.tile([128, LL], F16, tag=f"oc{k}", name=f"oc{k}")
                            nc.sync.dma_start(t[:], ag_out[k * 128:(k + 1) * 128, :])
                            oc_t.append(t)
                        wo_t = []
                        for k in range(NDD):
                            t = wop.tile([128, JC], F16, tag=f"wo{k}", name=f"wo{k}")
                            nc.sync.dma_start(t[:], wo[k * 128:(k + 1) * 128, :])
                            wo_t.append(t)

                        for m in range(NH):
                            pss = [yps.tile([128, 512], F32, tag=f"y{n}", name=f"y{n}")
                                   for n in range(NLC)]
                            for k in range(NDD):
                                for n in range(NLC):
                                    nc.tensor.matmul(
                                        pss[n][:],
                                        wo_t[k][:, m * 128:(m + 1) * 128],
                                        oc_t[k][:, n * 512:(n + 1) * 512],
                                        start=(k == 0), stop=(k == NDD - 1))
                            for n in range(NLC):
                                ysb = ysbp.tile([128, 512], F32, tag="ysb", name="ysb")
                                nc.scalar.activation(
                                    ysb[:], pss[n][:],
                                    mybir.ActivationFunctionType.Identity,
                                    bias=bo_t[:, m:m + 1], scale=1.0)
                                nc.sync.dma_start(
                                    yT[m * 128:(m + 1) * 128, n * 512:(n + 1) * 512],
                                    ysb[:])

    if compile:
        nc.compile()
    return nc


def _get(seq_len=L, repeat=1, with_cc=True):
    key = (seq_len, repeat, with_cc)
    if key not in _CACHE:
        _CACHE[key] = _build(seq_len, repeat=repeat, with_cc=with_cc)
    return _CACHE[key]


def _prep_inputs(x_real, x_imag, wq_r, wq_i, wk_r, wk_i, wv_r, wv_i,
                 wo_r, wo_i, bo_r, bo_i):
    """Host-side sharding: per-core input maps (fp16 layout prep)."""
    f16 = np.float16
    seq_len = x_real.shape[1]

    xcT_b = []
    for b in range(B):
        xcT_b.append(np.ascontiguousarray(
            np.concatenate([x_real[b].T, x_imag[b].T], axis=0)).astype(f16))

    mask01 = np.triu(np.ones((128, 128), dtype=np.float32)).astype(f16)
    ident = np.eye(128, dtype=np.float32).astype(f16)
    ones = np.ones((128, 1), dtype=f16)

    def proj_eff(w_r, w_i, heads):
        """[DD, 128*len(heads)] fp16: per head [r-cols(64) | i-cols(64)]."""
        w_eff = np.empty((DD, 128 * len(heads)), dtype=np.float32)
        for t, h in enumerate(heads):
            c0 = t * 128
            wr = w_r[64 * h:64 * h + 64, :].T    # [D, 64]
            wi = w_i[64 * h:64 * h + 64, :].T
            w_eff[:D, c0:c0 + 64] = wr
            w_eff[D:, c0:c0 + 64] = -wi
            w_eff[:D, c0 + 64:c0 + 128] = wi
            w_eff[D:, c0 + 64:c0 + 128] = wr
        return w_eff.astype(f16)

    in_maps = []
    for c in range(NCORES):
        b, g = divmod(c, GROUP)
        heads = [4 * g + t for t in range(NH)]
        ycols = slice(256 * g, 256 * g + 256)

        wq_eff = proj_eff(wq_r, wq_i, heads)
        wk_eff = proj_eff(wk_r, wk_i, heads)
        wv_eff = proj_eff(wv_r, wv_i, heads)

        # wo_eff rows follow the AllGather row order: for each global head hh:
        # 64 rows of o_r dims, then 64 rows of o_i dims.
        wo_eff = np.empty((DD, JC), dtype=np.float32)
        for hh in range(H):
            dr = slice(64 * hh, 64 * hh + 64)
            r0 = 128 * hh
            wo_eff[r0:r0 + 64, 0:256] = wo_r[ycols, dr].T
            wo_eff[r0 + 64:r0 + 128, 0:256] = -wo_i[ycols, dr].T
            wo_eff[r0:r0 + 64, 256:512] = wo_i[ycols, dr].T
            wo_eff[r0 + 64:r0 + 128, 256:512] = wo_r[ycols, dr].T
        wo_eff = wo_eff.astype(f16)

        bo_eff = np.concatenate(
            [bo_r[ycols], bo_i[ycols]]).astype(np.float32).reshape(JC, 1)

        in_maps.append({
            "xcT": xcT_b[b], "wq": wq_eff, "wk": wk_eff, "wv": wv_eff,
            "wo": wo_eff, "bo": bo_eff, "mask": mask01, "ident": ident,
            "ones": ones,
        })
    return in_maps, seq_len


def _run(in_maps, seq_len):
    nc = _get(seq_len)
    res = run_bass_kernel_spmd(nc, in_maps, core_ids=list(range(NCORES)),
                               trace=False)
    return res


def _assemble(results, seq_len):
    yr = np.empty((B, seq_len, D), dtype=np.float32)
    yi = np.empty((B, seq_len, D), dtype=np.float32)
    for c in range(NCORES):
        b, g = divmod(c, GROUP)
        yT_c = results[c]["yT"]                      # [512, LL]
        yr[b][:, 256 * g:256 * g + 256] = yT_c[:256].T
        yi[b][:, 256 * g:256 * g + 256] = yT_c[256:].T
    return yr, yi


def kernel(x_real, x_imag, wq_r, wq_i, wk_r, wk_i, wv_r, wv_i,
           wo_r, wo_i, bo_r, bo_i):
    args = [np.asarray(a) for a in (x_real, x_imag, wq_r, wq_i, wk_r, wk_i,
                                    wv_r, wv_i, wo_r, wo_i, bo_r, bo_i)]
    in_maps, seq_len = _prep_inputs(*args)
    res = _run(in_maps, seq_len)
    return _assemble(res.results, seq_len)



# revision 25
# speedup vs baseline: 1.2718x; 1.2718x over previous
"""Complex-valued causal attention on 8 trn2 NeuronCores.

nn_ComplexAttention: B=2, L=2048, D=1024, H=16 heads (hd=64), fp32 I/O.

Sharding (per the batch+head hint): core c owns batch b = c//4 and heads
4*(c%4) .. 4*(c%4)+3.  Data parallel over B (2 groups of 4 cores), tensor
parallel over heads within a group.  After per-head attention the 4 cores of
a group AllGather the (d-major, fp16) attention outputs and each computes a
256-column slice of the output projection.

v3: fully software-pipelined schedule with partition-major DRAM layouts.
Projections (A), attention (B) and output projection (C) are emitted
interleaved per 512-seq chunk:

    A0 B0 A1 B1 A2 B2 C0 A3 B3 C1 C2 C3

so the PE never waits on the AllGather (one collective per q-chunk, issued
as soon as that chunk's attention outputs land) and ScalarE's exp work
overlaps the projection matmuls.  All bulk tensors are host-prepped
partition-major ([128, k, cols]) so each weight / x-chunk / y-chunk moves
in ONE large DMA — the whole kernel issues ~50 DMAs instead of ~230,
keeping the (serialized) DGE issue path off the critical path.  Softmax
denominators come from a DVE running sum + one 1-row matmul per (head,
q-chunk); the 1/sum broadcast is a K=1 PE outer product.  V is computed
seq-major directly (x as the stationary operand), no PE transposes.
ScalarE runs only Exp so its activation table is loaded exactly once.

All on-chip math uses fp16 operands with fp32 PSUM accumulation.  The
complex arithmetic (4 real matmuls per complex one) is folded into the
host-assembled W_eff matrices with +-W_r/W_i blocks:

  Qc^T[h] = Wq_eff^T @ xc^T        (xc^T = [x_real^T ; x_imag^T], host-prepped)
  S^T     = Kc^T-block^T @ Qc^T    (real part of complex dot product, both
                                    r/i folded into the 128-deep contraction)
  w^T     = exp(SCALE * S^T)       (no max-subtraction needed: |scores| <~ 8)
  O^T     = V-block^T @ w^T        (V seq-major, computed directly)
  y^T     = Wo_eff^T @ oc^T        (oc^T = per-chunk AllGather of all heads)
"""

import sys

if "/opt/trn_rl_repo" not in sys.path:
    sys.path.insert(0, "/opt/trn_rl_repo")

import numpy as np
import ml_dtypes

import concourse.mybir as mybir
import concourse.tile as tile
from concourse import bacc
from concourse.bass_utils import run_bass_kernel_spmd

B, L, D, H = 2, 2048, 1024, 16
HD = D // H            # 64
SCALE = HD ** (-0.5)
NCORES = 8
GROUP = 4              # cores per batch group
NH = H // GROUP        # 4 local heads per core
JC = NH * 2 * HD       # 512 local projection cols (r+i interleaved by head)
DD = 2 * D             # 2048 stacked (real; imag) contraction dim
NDD = DD // 128        # 16 contraction chunks
F16 = mybir.dt.float16
F32 = mybir.dt.float32

_CACHE = {}


def _build(seq_len=L, repeat=1, with_cc=True, compile=True):
    """Build + compile the SPMD kernel (identical program on all 8 cores).

    repeat>1 wraps the whole body in a hardware For_i loop (timing variant,
    collective skipped since collectives cannot sit inside control flow).
    """
    from contextlib import nullcontext
    LL = seq_len
    NLC = LL // 512        # q/seq chunks of 512
    NKB = LL // 128        # k/seq blocks of 128

    nc = bacc.Bacc("TRN2", target_bir_lowering=False, debug=False,
                   num_devices=NCORES)

    # all bulk tensors partition-major: [128, chunk, cols]
    xcT = nc.dram_tensor("xcT", [128, NDD, LL], F16, kind="ExternalInput")
    wq = nc.dram_tensor("wq", [128, NDD, JC], F16, kind="ExternalInput")
    wk = nc.dram_tensor("wk", [128, NDD, JC], F16, kind="ExternalInput")
    wv = nc.dram_tensor("wv", [128, NDD, JC], F16, kind="ExternalInput")
    wo = nc.dram_tensor("wo", [128, NDD, JC], F16, kind="ExternalInput")
    bo = nc.dram_tensor("bo", [128, NH], F32, kind="ExternalInput")
    mask = nc.dram_tensor("mask", [128, 128], F16, kind="ExternalInput")
    ones = nc.dram_tensor("ones", [128, 128], F16, kind="ExternalInput")
    yT = nc.dram_tensor("yT", [128, NH, LL], F32, kind="ExternalOutput")

    with tile.TileContext(nc) as tc:
        with (
            tc.tile_pool(name="const", bufs=1) as const,
            tc.tile_pool(name="dram", bufs=1, space="DRAM") as dram,
        ):
            mask_t = const.tile([128, 128], F16, tag="mask", name="mask")
            ones_t = const.tile([128, 128], F16, tag="ones", name="ones")
            bo_t = const.tile([128, NH], F32, tag="bo", name="bo")

            def load_consts():
                nc.sync.dma_start(mask_t[:], mask[:])
                nc.sync.dma_start(ones_t[:], ones[:])
                nc.sync.dma_start(bo_t[:], bo[:])

            if repeat > 1:
                load_consts()

            # per-chunk collective staging: head-major in the free dim so one
            # group's heads move in a single 512KB DMA.
            ag_in = [dram.tile([128, NH * 512], F16, tag=f"agi{qc}",
                               name=f"agi{qc}") for qc in range(NLC)]
            ag_out = [dram.tile([GROUP * 128, NH * 512], F16, tag=f"ago{qc}",
                                name=f"ago{qc}") for qc in range(NLC)]

            _hint = (mybir.EngineType.PE, mybir.EngineType.Activation,
                     mybir.EngineType.DVE, mybir.EngineType.SP,
                     mybir.EngineType.Pool)
            loop_cm = (tc.For_i(0, repeat, 1, hint_engines=_hint)
                       if repeat > 1 else nullcontext())
            with loop_cm:
                with (
                    tc.tile_pool(name="wqkv", bufs=1) as wqkv,
                    tc.tile_pool(name="xp", bufs=2) as xp,
                    tc.tile_pool(name="qkp", bufs=1) as qkp,
                    tc.tile_pool(name="vp", bufs=1) as vp,
                    tc.tile_pool(name="wxp", bufs=6) as wxp,
                    tc.tile_pool(name="wsp", bufs=2) as wsp,
                    tc.tile_pool(name="epi", bufs=3) as epi,
                    tc.tile_pool(name="ocp", bufs=2) as ocp,
                    tc.tile_pool(name="ysbp", bufs=1) as ysbp,
                    tc.tile_pool(name="pps", bufs=2, space="PSUM") as pps,
                    tc.tile_pool(name="sps", bufs=3, space="PSUM") as sps,
                    tc.tile_pool(name="ops", bufs=2, space="PSUM") as ops,
                    tc.tile_pool(name="sums", bufs=1, space="PSUM") as sums,
                ):
                    # ---------- bulk DMAs, four per tensor/chunk ----------
                    # quarter-tiles (512KB) so the first accumulation chains
                    # start almost immediately while staying far under the
                    # per-DMA overhead budget.
                    NQ = 4
                    HK = NDD // NQ
                    w_t = {}
                    x_t = [None] * NLC

                    def load_w(wname, wext, part=None):
                        parts = w_t.setdefault(wname, [None] * NQ)
                        rng = range(NQ) if part is None else [part]
                        for i in rng:
                            t = wqkv.tile([128, HK, JC], F16,
                                          tag=f"{wname}{i}",
                                          name=f"{wname}{i}")
                            nc.sync.dma_start(
                                t[:], wext[:, i * HK:(i + 1) * HK, :])
                            parts[i] = t

                    def load_x(n, part=None):
                        if x_t[n] is None:
                            x_t[n] = [None] * NQ
                        rng = range(NQ) if part is None else [part]
                        for i in rng:
                            t = xp.tile([128, HK, 512], F16, tag=f"x{i}",
                                        name=f"x{n}_{i}")
                            nc.sync.dma_start(
                                t[:], xcT[:, i * HK:(i + 1) * HK,
                                          n * 512:(n + 1) * 512])
                            x_t[n][i] = t

                    def wslice(wname, k, c0, c1):
                        return w_t[wname][k // HK][:, k % HK, c0:c1]

                    def xslice(n, k, c0, c1):
                        return x_t[n][k // HK][:, k % HK, c0:c1]

                    for i in range(NQ):
                        load_w("wq", wq, i)
                        load_x(0, i)
                        if i == 0 and repeat == 1:
                            load_consts()
                    load_w("wk", wk)
                    load_w("wv", wv)
                    load_x(1)
                    load_w("wo", wo)

                    qT = [[None] * NLC for _ in range(NH)]
                    kT = [[None] * NLC for _ in range(NH)]
                    vv = [None] * NKB

                    # ---------- phase emitters ----------
                    def gen_A(n):
                        """A(n) as a generator: yields after every 4-matmul
                        segment so B streams can interleave filler PE work."""
                        def qk_chains():
                            for wname, dest in (("wq", qT), ("wk", kT)):
                                for h in range(NH):
                                    yield (
                                        lambda k, ps, wname=wname, h=h:
                                        nc.tensor.matmul(
                                            ps[:],
                                            wslice(wname, k, h * 128,
                                                   (h + 1) * 128),
                                            xslice(n, k, 0, 512),
                                            start=(k == 0),
                                            stop=(k == NDD - 1)),
                                        (qkp, f"{wname}T{h}_{n}", dest, h))
                            for j in range(4):
                                yield (
                                    lambda k, ps, j=j: nc.tensor.matmul(
                                        ps[:],
                                        xslice(n, k, j * 128, (j + 1) * 128),
                                        wslice("wv", k, 0, JC),
                                        start=(k == 0), stop=(k == NDD - 1)),
                                    (vp, f"vv{4 * n + j}", vv, 4 * n + j))

                        for mm, (pool, tag, dest, di) in qk_chains():
                            ps = pps.tile([128, 512], F32, tag="proj",
                                          name="proj")
                            for k in range(NDD):
                                mm(k, ps)
                                if k % 4 == 3:
                                    if k == NDD - 1:
                                        t = pool.tile([128, 512], F16,
                                                      tag=tag, name=tag)
                                        nc.vector.tensor_copy(t[:], ps[:])
                                        if dest is vv:
                                            vv[di] = t
                                        else:
                                            dest[di][n] = t
                                    yield

                    oc_t = [None] * NLC

                    def emit_B(qc, filler=None, fill_every=4):
                        nkb = 4 * (qc + 1)
                        nfill = [0]

                        def fill():
                            nfill[0] += 1
                            if filler is not None and nfill[0] % fill_every == 0:
                                next(filler, None)

                        epi_pend = [None]

                        def epilogue():
                            if epi_pend[0] is None:
                                return
                            h, o_ps, wsum = epi_pend[0]
                            epi_pend[0] = None
                            # softmax denominators: all-ones [128,128]
                            # stationary -> every PSUM row holds the column
                            # sums (one 213ns matmul, broadcast included),
                            # then a single DVE reciprocal into SBUF.
                            wsum16 = wsp.tile([128, 512], F16, tag="ws16",
                                              name="ws16")
                            nc.vector.tensor_copy(wsum16[:], wsum[:])
                            s_sum = sums.tile([128, 512], F32, tag="ssum",
                                              name="ssum")
                            nc.tensor.matmul(s_sum[:], ones_t[:], wsum16[:],
                                             start=True, stop=True)
                            rec = epi.tile([128, 512], F16, tag="rec",
                                           name="rec")
                            with nc.allow_low_precision("f16 1/sums is 2^-11"):
                                nc.vector.reciprocal(rec[:], s_sum[:])
                            osc = epi.tile([128, 512], F16, tag="osc",
                                           name="osc")
                            nc.vector.tensor_mul(osc[:], o_ps[:], rec[:])
                            nc.sync.dma_start(
                                ag_in[qc][:, h * 512:(h + 1) * 512], osc[:])

                        for h in range(NH):
                            o_ps = ops.tile([128, 512], F32, tag="o", name="o")
                            wsum = wsp.tile([128, 512], F32, tag="ws", name="ws")
                            acc = {"init": False, "carry": None}

                            # running softmax-denominator sum: full-width
                            # blocks are pair-summed in f16 (2x DVE rate)
                            # before hitting the f32 accumulator.
                            def sink(t):
                                if not acc["init"]:
                                    nc.vector.tensor_copy(wsum[:], t[:])
                                    acc["init"] = True
                                else:
                                    nc.vector.tensor_add(wsum[:], wsum[:],
                                                         t[:])

                            def add_wx(f0, w, wx):
                                if f0 > 0:
                                    nc.vector.tensor_add(
                                        wsum[:, f0:512], wsum[:, f0:512],
                                        wx[:, :w])
                                elif not acc["init"]:
                                    sink(wx)
                                elif acc["carry"] is None:
                                    acc["carry"] = wx
                                else:
                                    pair = epi.tile([128, 512], F16,
                                                    tag="pair", name="pair")
                                    nc.vector.tensor_add(
                                        pair[:], acc["carry"][:], wx[:])
                                    acc["carry"] = None
                                    sink(pair)

                            # 2-stage software pipeline: issue the next two
                            # k-blocks' scores matmuls before the current
                            # block's O matmul so the PE rides out the exp
                            # latency on ScalarE.
                            def consume(kb, f0, w, wx):
                                nc.tensor.matmul(
                                    o_ps[:, f0:512],
                                    vv[kb][:, h * 128:(h + 1) * 128],
                                    wx[:, :w],
                                    start=(kb == 0), stop=(kb == nkb - 1))
                                add_wx(f0, w, wx)

                            pend = []
                            for kb in range(nkb):
                                r = kb - 4 * qc
                                f0 = 128 * r if r >= 0 else 0
                                w = 512 - f0
                                s_ps = sps.tile([128, 512], F32, tag="sc",
                                                name="sc")
                                nc.tensor.matmul(
                                    s_ps[:, :w],
                                    kT[h][kb // 4][:, (kb % 4) * 128:
                                                   (kb % 4 + 1) * 128],
                                    qT[h][qc][:, f0:512],
                                    start=True, stop=True)
                                wx = wxp.tile([128, 512], F16, tag="wx",
                                              name="wx")
                                nc.scalar.activation(
                                    wx[:, :w], s_ps[:, :w],
                                    mybir.ActivationFunctionType.Exp,
                                    scale=SCALE)
                                if r >= 0:
                                    nc.vector.tensor_mul(
                                        wx[:, :128], wx[:, :128], mask_t[:])
                                if kb == 1:
                                    # previous head's epilogue, off this
                                    # head's critical path
                                    epilogue()
                                if len(pend) >= 2:
                                    consume(*pend.pop(0))
                                pend.append((kb, f0, w, wx))
                                fill()
                            for p in pend:
                                consume(*p)
                            if acc["carry"] is not None:
                                sink(acc["carry"])
                            epi_pend[0] = (h, o_ps, wsum)
                        epilogue()

                        if with_cc:
                            nc.gpsimd.collective_compute(
                                "AllGather",
                                mybir.AluOpType.bypass,
                                replica_groups=[[0, 1, 2, 3], [4, 5, 6, 7]],
                                ins=[ag_in[qc].opt()],
                                outs=[ag_out[qc].opt()],
                            )
                        # oc load for this chunk: one 512KB DMA per peer
                        # group-rank, issued now so it fires as soon as the
                        # collective lands (long before C(qc) needs it).
                        tl = []
                        for g in range(GROUP):
                            t = ocp.tile([128, NH * 512], F16, tag=f"oc{g}",
                                         name=f"oc{qc}_{g}")
                            nc.sync.dma_start(
                                t[:], ag_out[qc][g * 128:(g + 1) * 128, :])
                            tl.append(t)
                        oc_t[qc] = tl

                    def gen_C(qc):
                        ysb = ysbp.tile([128, NH, 512], F32, tag="ysb",
                                        name="ysb")
                        for m in range(NH):
                            ps = pps.tile([128, 512], F32, tag="proj",
                                          name="proj")
                            for k in range(NDD):
                                nc.tensor.matmul(
                                    ps[:],
                                    wslice("wo", k, m * 128, (m + 1) * 128),
                                    oc_t[qc][k // 4][:, (k % 4) * 512:
                                                     (k % 4 + 1) * 512],
                                    start=(k == 0), stop=(k == NDD - 1))
                                if k % 4 == 3:
                                    if k == NDD - 1:
                                        nc.vector.tensor_scalar_add(
                                            ysb[:, m, :], ps[:],
                                            bo_t[:, m:m + 1])
                                        nc.sync.dma_start(
                                            yT[:, m, qc * 512:(qc + 1) * 512],
                                            ysb[:, m, :])
                                    yield

                    # ---------- pipelined schedule ----------
                    # B(qc) streams interleave one 4-matmul segment of the
                    # next A/C phase per 4 attention blocks, so the in-order
                    # PE queue carries ready filler work through exp stalls.
                    def drain(g):
                        for _ in g:
                            pass

                    def scoped(label, fn, *a):
                        with nc.named_scope(label):
                            return fn(*a)

                    scoped("A0", lambda: drain(gen_A(0)))
                    gf = gen_A(1)
                    scoped("B0", emit_B, 0, gf)
                    scoped("A1", lambda: drain(gf))
                    scoped("x2", load_x, 2)
                    gf = gen_A(2)
                    scoped("B1", emit_B, 1, gf)
                    scoped("A2", lambda: drain(gf))
                    scoped("x3", load_x, 3)
                    gf = gen_A(3)
                    scoped("B2", emit_B, 2, gf)
                    scoped("A3", lambda: drain(gf))
                    scoped("C0", lambda: drain(gen_C(0)))
                    gf = gen_C(1)
                    scoped("B3", emit_B, 3, gf)
                    scoped("C1", lambda: drain(gf))
                    scoped("C2", lambda: drain(gen_C(2)))
                    scoped("C3", lambda: drain(gen_C(3)))

    if compile:
        nc.compile()
    return nc


def _get(seq_len=L, repeat=1, with_cc=True):
    key = (seq_len, repeat, with_cc)
    if key not in _CACHE:
        _CACHE[key] = _build(seq_len, repeat=repeat, with_cc=with_cc)
    return _CACHE[key]


def _pmajor(a):
    """[NDD*128, C] -> [128, NDD, C] partition-major fp16."""
    n = a.shape[0] // 128
    return np.ascontiguousarray(
        a.reshape(n, 128, a.shape[1]).transpose(1, 0, 2)).astype(np.float16)


def _prep_inputs(x_real, x_imag, wq_r, wq_i, wk_r, wk_i, wv_r, wv_i,
                 wo_r, wo_i, bo_r, bo_i):
    """Host-side sharding: per-core input maps (fp16 layout prep)."""
    f16 = np.float16
    seq_len = x_real.shape[1]

    xcT_b = []
    for b in range(B):
        xcT_b.append(_pmajor(
            np.concatenate([x_real[b].T, x_imag[b].T], axis=0)))

    mask01 = np.triu(np.ones((128, 128), dtype=np.float32)).astype(f16)
    ones = np.ones((128, 128), dtype=f16)

    def proj_eff(w_r, w_i, heads):
        """[DD, 128*len(heads)]: per head [r-cols(64) | i-cols(64)]."""
        w_eff = np.empty((DD, 128 * len(heads)), dtype=np.float32)
        for t, h in enumerate(heads):
            c0 = t * 128
            wr = w_r[64 * h:64 * h + 64, :].T    # [D, 64]
            wi = w_i[64 * h:64 * h + 64, :].T
            w_eff[:D, c0:c0 + 64] = wr
            w_eff[D:, c0:c0 + 64] = -wi
            w_eff[:D, c0 + 64:c0 + 128] = wi
            w_eff[D:, c0 + 64:c0 + 128] = wr
        return w_eff

    in_maps = []
    for c in range(NCORES):
        b, g = divmod(c, GROUP)
        heads = [4 * g + t for t in range(NH)]
        ycols = slice(256 * g, 256 * g + 256)

        wq_eff = proj_eff(wq_r, wq_i, heads)
        wk_eff = proj_eff(wk_r, wk_i, heads)
        wv_eff = proj_eff(wv_r, wv_i, heads)

        # wo_eff rows follow the AllGather row order: contraction chunk k is
        # global head k: [64 rows of o_r dims; 64 rows of o_i dims].
        wo_eff = np.empty((DD, JC), dtype=np.float32)
        for hh in range(H):
            dr = slice(64 * hh, 64 * hh + 64)
            r0 = 128 * hh
            wo_eff[r0:r0 + 64, 0:256] = wo_r[ycols, dr].T
            wo_eff[r0 + 64:r0 + 128, 0:256] = -wo_i[ycols, dr].T
            wo_eff[r0:r0 + 64, 256:512] = wo_i[ycols, dr].T
            wo_eff[r0 + 64:r0 + 128, 256:512] = wo_r[ycols, dr].T

        bo_eff = np.concatenate(
            [bo_r[ycols], bo_i[ycols]]).astype(np.float32)

        in_maps.append({
            "xcT": xcT_b[b],
            "wq": _pmajor(wq_eff), "wk": _pmajor(wk_eff),
            "wv": _pmajor(wv_eff), "wo": _pmajor(wo_eff),
            "bo": np.ascontiguousarray(
                bo_eff.reshape(NH, 128).T).astype(np.float32),
            "mask": mask01, "ones": ones,
        })
    return in_maps, seq_len


def _run(in_maps, seq_len):
    nc = _get(seq_len)
    res = run_bass_kernel_spmd(nc, in_maps, core_ids=list(range(NCORES)),
                               trace=False)
    return res


def _assemble(results, seq_len):
    yr = np.empty((B, seq_len, D), dtype=np.float32)
    yi = np.empty((B, seq_len, D), dtype=np.float32)
    for c in range(NCORES):
        b, g = divmod(c, GROUP)
        yT_c = results[c]["yT"]                      # [128, NH, LL]
        yT_c = yT_c.transpose(1, 0, 2).reshape(JC, seq_len)
        yr[b][:, 256 * g:256 * g + 256] = yT_c[:256].T
        yi[b][:, 256 * g:256 * g + 256] = yT_c[256:].T
    return yr, yi


def kernel(x_real, x_imag, wq_r, wq_i, wk_r, wk_i, wv_r, wv_i,
           wo_r, wo_i, bo_r, bo_i):
    args = [np.asarray(a) for a in (x_real, x_imag, wq_r, wq_i, wk_r, wk_i,
                                    wv_r, wv_i, wo_r, wo_i, bo_r, bo_i)]
    in_maps, seq_len = _prep_inputs(*args)
    res = _run(in_maps, seq_len)
    return _assemble(res.results, seq_len)
